# revision 26
# baseline (speedup 1.0000x reference)
"""Trainium2 Bass kernel for nn_AllTimes (sort 4000 times/row -> adjacent
diffs -> mask by N_total).

Self-contained: kernel(**inputs) takes the FULL inputs
  input_times [5, 256, 800] f32, N_total [256] int (int32/int64)
and returns the FULL output [256, 3995, 1] f32.

Strategy: pure data parallel across 8 NeuronCores (32 batch rows each).
Per core, the 32 rows' 4000 values (padded to 4096 with BIG) are laid out
as [128 partitions, 1024] f32 (partition QP[s] + r holds row r's segment s)
and sorted with a bitonic merge network (78 half-stages) at TWO elements
per cycle per lane: the stock TENSOR_SCALAR_ARITH_OP (0x43) DVE table row
is overridden so its 2X_2PORT perf-mode slot holds a pairwise
compare-exchange uop program (port0 = first half of the AP stream, port1 =
second half; min -> WR0/port0 position, max -> WR1/port1 position). A
single-src fp32 SBUF->SBUF tensor_scalar auto-selects 2X_2PORT, so one
instruction with APs rearranged "p (b two j) -> p two b j" executes one
half-stage at stride j in ~(j_total/2)/0.96GHz. Mirror stages are
eliminated by a descending-odd-runs invariant: each level's closing j=1
stage writes odd runs descending (two half-width ops via the
flat-reversed view), making every level's first stage a straight riffle
on an asc-desc (bitonic) input. Cross-segment stages co-locate pair
members in cols [L, 2L) of the same partitions (copy + one flat-[2L]
hijack op + copy-back). Adjacent diffs and the N_total mask are fused
on-device (MASKED_DIFF custom op); the host only reshapes/concatenates
shards. The timing loop double-buffers two tile sets (16-body unroll)
with input DMAs on the ACT queue and output DMA on the SP queue so all
DMA hides behind DVE compute.
"""

import sys

sys.path.insert(0, "/opt/trn_rl_repo")

from contextlib import ExitStack

import numpy as np

import concourse.bass as bass
import concourse.bacc as bacc
import concourse.mybir as mybir
from concourse.tile import TileContext
from concourse import bass_utils

FP32 = mybir.dt.float32
AL = mybir.AluOpType
MIN = AL.min
MAX = AL.max


# ---------------------------------------------------------------------------
# fused compare-exchange custom DVE op:
#   out[k] = k < s0 ? min(Src0[k], Src1[k]) : max(Src0[k], Src1[k])
# One instruction computes both halves of a bitonic half-stage (for the
# stage shapes whose APs fit the custom-op 2-free-dim limit).
# ---------------------------------------------------------------------------
def _register_cmpxchg():
    import concourse.dve_ops as dve_ops
    from concourse.dve_spec import Spec, Src0, Src1, C0, Idx, minn, maxx, select, lower
    from concourse.dve_uop import DveOpSpec

    name = "CMPXCHG_HALVES_ANT"
    if name in dve_ops._SUB_OPCODE_FOR_NAME:
        return next(op for op in dve_ops.OPS if op.name == name)

    def _ref(in0, in1, s0, s1, imm2):
        x0 = np.asarray(in0, dtype=np.float32)
        x1 = np.asarray(in1, dtype=np.float32)
        P = x0.shape[0]
        f0 = x0.reshape(P, -1)
        f1 = x1.reshape(P, -1)
        idx = np.arange(f0.shape[1], dtype=np.float32)[None, :]
        thr = np.asarray(s0, dtype=np.float32).reshape(-1, 1)
        r = np.where(idx < thr, np.minimum(f0, f1), np.maximum(f0, f1))
        return r.reshape(x0.shape).astype(np.float32)

    spec = Spec(
        body=select(Idx < C0, minn(Src0, Src1), maxx(Src0, Src1)), reference=_ref
    )
    opcode = dve_ops._CUSTOM_DVE_ROW_BASE + len(dve_ops.OPS)
    shas = {}
    for ver in ("v3", "v4"):
        try:
            shas[ver] = DveOpSpec(
                name=name, opcode=opcode, uops=lower(spec, ver=ver), rd1_en=True
            ).sha(ver)
        except Exception:
            pass
    op = dve_ops.DveOp(name, spec, subdim=False, uops_sha=shas)
    dve_ops.OPS.append(op)
    dve_ops.CUSTOM_DVE_SPECS[name] = spec
    dve_ops._SUB_OPCODE_FOR_NAME[name] = opcode
    return op


CMPXCHG = _register_cmpxchg()


def _register_mirror_paged():
    """Paged mirror compare-exchange: for [P, nb, tm] streams,
    out[k] = (Idx < m + page*tm) ? min : max — i.e. within each tm-page,
    first half mins, second half maxes. s0 = m, s1 = tm."""
    import concourse.dve_ops as dve_ops
    from concourse.dve_spec import (
        Spec,
        Src0,
        Src1,
        C0,
        C1,
        Idx,
        PageIdx,
        minn,
        maxx,
        select,
        lower,
    )
    from concourse.dve_uop import DveOpSpec

    name = "CMPX_MIRROR_PAGED_ANT"
    if name in dve_ops._SUB_OPCODE_FOR_NAME:
        return next(op for op in dve_ops.OPS if op.name == name)

    def _ref(in0, in1, s0, s1, imm2):
        x0 = np.asarray(in0, dtype=np.float32)
        x1 = np.asarray(in1, dtype=np.float32)
        P = x0.shape[0]
        tm = int(x0.shape[-1])
        f0 = x0.reshape(P, -1, tm)
        f1 = x1.reshape(P, -1, tm)
        m = int(np.asarray(s0).flat[0])
        k = np.arange(tm)[None, None, :]
        r = np.where(k < m, np.minimum(f0, f1), np.maximum(f0, f1))
        return r.reshape(x0.shape).astype(np.float32)

    spec = Spec(
        body=select(
            Idx < PageIdx(C0, C1), minn(Src0, Src1), maxx(Src0, Src1)
        ),
        reference=_ref,
    )
    opcode = dve_ops._CUSTOM_DVE_ROW_BASE + len(dve_ops.OPS)
    shas = {}
    for ver in ("v3", "v4"):
        try:
            shas[ver] = DveOpSpec(
                name=name, opcode=opcode, uops=lower(spec, ver=ver), rd1_en=True
            ).sha(ver)
        except Exception:
            pass
    op = dve_ops.DveOp(name, spec, subdim=True, uops_sha=shas)
    dve_ops.OPS.append(op)
    dve_ops.CUSTOM_DVE_SPECS[name] = spec
    dve_ops._SUB_OPCODE_FOR_NAME[name] = opcode
    return op


MIRROR_PAGED = _register_mirror_paged()


def _register_masked_diff():
    """Fused tail op: out[k] = (Idx < s0[p]) ? (Src0[k] - Src1[k]) : 0."""
    import concourse.dve_ops as dve_ops
    from concourse.dve_spec import Spec, Src0, Src1, C0, Idx, Zero, select, lower
    from concourse.dve_uop import DveOpSpec

    name = "MASKED_DIFF_ANT"
    if name in dve_ops._SUB_OPCODE_FOR_NAME:
        return next(op for op in dve_ops.OPS if op.name == name)

    def _ref(in0, in1, s0, s1, imm2):
        x0 = np.asarray(in0, dtype=np.float32)
        x1 = np.asarray(in1, dtype=np.float32)
        P = x0.shape[0]
        f0 = x0.reshape(P, -1)
        f1 = x1.reshape(P, -1)
        idx = np.arange(f0.shape[1], dtype=np.float32)[None, :]
        thr = np.asarray(s0, dtype=np.float32).reshape(-1, 1)
        r = np.where(idx < thr, f0 - f1, np.float32(0.0))
        return r.reshape(x0.shape).astype(np.float32)

    spec = Spec(body=select(Idx < C0, Src0 - Src1, Zero), reference=_ref)
    opcode = dve_ops._CUSTOM_DVE_ROW_BASE + len(dve_ops.OPS)
    shas = {}
    for ver in ("v3", "v4"):
        try:
            shas[ver] = DveOpSpec(
                name=name, opcode=opcode, uops=lower(spec, ver=ver), rd1_en=True
            ).sha(ver)
        except Exception:
            pass
    op = dve_ops.DveOp(name, spec, subdim=False, uops_sha=shas)
    dve_ops.OPS.append(op)
    dve_ops.CUSTOM_DVE_SPECS[name] = spec
    dve_ops._SUB_OPCODE_FOR_NAME[name] = opcode
    return op


MASKED_DIFF = _register_masked_diff()


# ---------------------------------------------------------------------------
# Hijacked TENSOR_SCALAR_ARITH_OP (0x43) row: the 2X_2PORT perf-mode slot
# holds a pairwise compare-exchange program. A single-src fp32 SBUF->SBUF
# tensor_scalar auto-selects 2X_2PORT: port0 streams the first half of the
# AP stream, port1 the second half; per cycle the program computes
# min(a,b) -> WR0 (port0 position) and max(a,b) -> WR1 (port1 position).
# With APs rearranged "p (b two j) -> p two b j" this is one bitonic
# half-stage at 2 elements/cycle (~594ns for 1024 cols vs ~1224ns at 1x).
# The REGULAR slot is a plain copy so an (unexpected) fallback is caught by
# the correctness check rather than producing garbage.
# ---------------------------------------------------------------------------
HIJACK_NAME = "TS_CMPXCHG_HIJACK_ANT"
TS_OPCODE = 0x43  # TENSOR_SCALAR_ARITH_OP


def _register_ts_hijack():
    import concourse.dve_ops as dve_ops
    from concourse.dve_spec import Spec, Src0
    from concourse.dve_uop import (
        UopConfig,
        DveOpSpec,
        AluOp,
        AluInp,
        DelayInp,
        InpSel,
        OutSel,
        OutPath,
        Trigger,
        ENABLE,
    )

    if any(op.name == HIJACK_NAME for op in dve_ops.OPS):
        return

    def regular_copy():
        u = UopConfig()
        u.enable_input(InpSel.SRC_0, 0)
        u.require_inp0 = ENABLE
        u.trigger = (Trigger.SRC_TENSOR_DONE, Trigger.NONE, Trigger.NONE)
        u.enable_rev_ops = ENABLE
        for b in range(8):
            u.datapath_config[b].pass_through_alu()
        u.enable_output(OutSel.ALU_OUT, OutPath.WR0_LO)
        return u

    def pairwise():
        u = UopConfig()
        u.enable_input(InpSel.SRC_0, 0)
        u.enable_input(InpSel.SRC_1, 1)
        u.require_inp0 = ENABLE
        u.require_inp1 = ENABLE
        u.trigger = (Trigger.SRC_TENSOR_DONE, Trigger.NONE, Trigger.NONE)
        u.enable_rev_ops = ENABLE
        dp = u.datapath_config
        dp[0].enable_alu(AluOp.MIN, AluInp.PREV_ALU_OUT, AluInp.PREV_DELAY_0)
        dp[0].enable_delay_from_src(DelayInp.PREV_DELAY, 0)  # chain0 <- src1
        dp[0].enable_delay_from_src(DelayInp.PREV_ALU_OUT, 1)  # chain1 <- src0
        dp[1].enable_alu(AluOp.MAX, AluInp.PREV_DELAY_1, AluInp.PREV_DELAY_0)
        dp[1].enable_delay_from_src(DelayInp.PREV_ALU_OUT, 2)  # chain2 <- min
        dp[2].pass_through_delay(2)
        dp[2].enable_delay_from_src(DelayInp.PREV_ALU_OUT, 3)  # chain3 <- max
        for b in range(3, 8):
            dp[b].pass_through_delay(2, 3)
        u.enable_output(OutSel.DELAY_2, OutPath.WR0_LO)  # min
        u.enable_output(OutSel.DELAY_3, OutPath.WR1_LO)  # max
        return u

    spec = DveOpSpec(
        name=HIJACK_NAME,
        opcode=TS_OPCODE,
        uops=[regular_copy()],
        uops_2x=[regular_copy()],
        uops_2x_2p=[pairwise()],
        uops_4x=None,
    )

    def _ref(in0, in1, s0, s1, imm2):
        return np.asarray(in0, dtype=np.float32)

    entry = dve_ops.DveOp(
        HIJACK_NAME, Spec(body=Src0, reference=_ref), subdim=False, uops_sha={}
    )
    dve_ops.OPS.append(entry)
    dve_ops.CUSTOM_DVE_SPECS[HIJACK_NAME] = entry.spec
    for ver in ("v3", "v4"):
        dve_ops._COMPILE_CACHE[(HIJACK_NAME, ver)] = spec


_register_ts_hijack()

N_CORES = 8
NBANDS = 5
BANDLEN = 800
ROWS = 32  # batch rows per core
L = 1024  # elements per partition (segment length); 4 segs per row
NOUT = 3995
BIG = 3.0e38

# Logical segment s lives at partition offset QP[s] (quadrant placement
# [s0, s2, s1, s3]) so that the level-A and level-B-stride cross-segment
# half-stages become single 64-partition ops.
QP = [0, 64, 32, 96]

# (band k, t0, t1, seg s, j0): input_times[k, :, t0:t1] -> X[QP[s]:+32, j0:...]
PIECES = [
    (0, 0, 800, 0, 0),
    (1, 0, 224, 0, 800),
    (1, 224, 800, 1, 0),
    (2, 0, 448, 1, 576),
    (2, 448, 800, 2, 0),
    (3, 0, 672, 2, 352),
    (3, 672, 800, 3, 0),
    (4, 0, 800, 3, 128),
]


def _emit_split(nc, dve_frac, op, make_aps, width):
    """One logical compare op, split by columns across DVE and Pool."""
    c = int(round(width * dve_frac))
    c = max(0, min(width, c))
    if c > 0:
        out, a, b = make_aps(0, c)
        nc.vector.tensor_tensor(out=out, in0=a, in1=b, op=op)
    if c < width:
        out, a, b = make_aps(c, width)
        nc.gpsimd.tensor_tensor(out=out, in0=a, in1=b, op=op)


def _rev(base, lo, hi):
    stop = base - hi
    return slice(base - lo, (stop if stop >= 0 else None), -1)


def _down_powers(start, stop):
    j = start
    while j >= stop:
        yield j
        j //= 2


def emit_sort(nc, cur, nxt, seglen, dve_frac=1.0, cross_dve_frac=1.0, fused=True):
    """Sort network over [128, seglen] ping-pong tiles cur/nxt; T is a
    [128, seglen] scratch tile for cross-segment operand alignment.
    Returns the tile holding the sorted result."""
    L = seglen
    nlev = L.bit_length() - 1
    H = L // 2

    def halfstage(ops):
        nonlocal cur, nxt
        for op, make, width in ops:
            _emit_split(nc, dve_frac, op, make, width)
        cur, nxt = nxt, cur

    def fused_j1():
        # one instruction: mins to even cols, maxes to odd cols
        nonlocal cur, nxt
        lo = cur[:, 0::2].unsqueeze(1).to_broadcast((128, 2, H))
        hi = cur[:, 1::2].unsqueeze(1).to_broadcast((128, 2, H))
        o = nxt[:, :].rearrange("p (b two) -> p two b", two=2)
        nc.vector._custom_dve(CMPXCHG, out=o, in0=lo, in1=hi, s0=float(H))
        cur, nxt = nxt, cur

    def fused_j_half():
        # stride L/2 (single block): mins to cols [0,H), maxes to [H,L)
        nonlocal cur, nxt
        lo = cur[:, 0:H].unsqueeze(1).to_broadcast((128, 2, H))
        hi = cur[:, H:L].unsqueeze(1).to_broadcast((128, 2, H))
        nc.vector._custom_dve(CMPXCHG, out=nxt[:, :], in0=lo, in1=hi, s0=float(H))
        cur, nxt = nxt, cur

    def riffle_stage(jj, parts=slice(0, 128)):
        # One compare-exchange half-stage at stride jj as a single hijacked
        # tensor_scalar: halves-major AP [p, two, b, j]; 2X_2PORT computes
        # min -> lo position, max -> hi position at 2 elems/cycle.
        nonlocal cur, nxt
        nb = L // (2 * jj)
        if nb >= 2:
            # two interleaved half-ops (even/odd 2j-blocks): measured cheaper
            # than one full-width op (667 vs 736ns) - the independent strided
            # halves pipeline better on the DVE
            if jj == 1:
                ci = cur[parts, 0:L].rearrange(
                    "p (rh r2 two) -> p r2 two rh", r2=2, two=2
                )
                no = nxt[parts, 0:L].rearrange(
                    "p (rh r2 two) -> p r2 two rh", r2=2, two=2
                )
            else:
                ci = cur[parts, 0:L].rearrange(
                    "p (rh r2 two j) -> p r2 two rh j", r2=2, two=2, j=jj
                )
                no = nxt[parts, 0:L].rearrange(
                    "p (rh r2 two j) -> p r2 two rh j", r2=2, two=2, j=jj
                )
            nc.vector.tensor_scalar_add(no[:, 0], ci[:, 0], 0.0)
            nc.vector.tensor_scalar_add(no[:, 1], ci[:, 1], 0.0)
        else:
            i_ap = cur[parts, 0:L].rearrange(
                "p (b two j) -> p two b j", two=2, j=jj
            )
            o_ap = nxt[parts, 0:L].rearrange(
                "p (b two j) -> p two b j", two=2, j=jj
            )
            nc.vector.tensor_scalar_add(o_ap, i_ap, 0.0)
        cur, nxt = nxt, cur

    def split_j1_stage(m2):
        # Last (j=1) stage of the level finalizing m2-runs: write ODD m2-runs
        # descending so the next level's first stage is a straight riffle
        # (asc‖desc is bitonic). Two half-width ops: even runs via the normal
        # view, odd runs via the flat-reversed view (odd-run data appears at
        # the reversed view's r2=0 slot, and min->higher-address = descending
        # storage falls out of the same construction).
        nonlocal cur, nxt
        q = max(1, m2 // 2)
        ci = cur[:, 0:L].rearrange(
            "p (rh r2 q two) -> p r2 two rh q", r2=2, two=2, q=q
        )
        no = nxt[:, 0:L].rearrange(
            "p (rh r2 q two) -> p r2 two rh q", r2=2, two=2, q=q
        )
        rno = nxt[:, L - 1 :: -1].rearrange(
            "p (rh r2 q two) -> p r2 two rh q", r2=2, two=2, q=q
        )
        # even runs: normal pairs, ascending write
        nc.vector.tensor_scalar_add(no[:, 0], ci[:, 0], 0.0)
        # odd runs: input pairs with rh reversed (to match the reversed-view
        # output enumeration); min lands at local m2-1-2k => descending run
        nc.vector.tensor_scalar_add(rno[:, 0], ci[:, 1, :, ::-1, :], 0.0)
        cur, nxt = nxt, cur

    def uniform_stages(first_j):
        j = first_j
        while j >= 1:
            riffle_stage(j)
            j //= 2

    # ---- Phase 1: in-partition sort to runs of L -------------------------
    # Descending-odd-runs invariant: the j=1 stage closing each level writes
    # odd runs descending (split_j1_stage), so every level's first stage is a
    # straight riffle at j=m (asc‖desc input is bitonic) — no mirror ops.
    # Level 10 closes with a normal j=1 stage; level A's copy handles the
    # cross-segment reversal as before.
    for lev in range(1, nlev + 1):
        m = 1 << (lev - 1)
        if lev > 1:
            riffle_stage(m)  # straight first stage on asc‖desc runs
        for j in _down_powers(m // 2, 2):
            riffle_stage(j)
        if lev < nlev:
            split_j1_stage(2 * m)
        else:
            riffle_stage(1)

    # ---- cross-segment half-stages ---------------------------------------
    # With the QP placement ([s0@Q0, s2@Q1, s1@Q2, s3@Q3]), the level-A
    # mirror and level-B straight stages pair partitions 0:64 with 64:128,
    # so each is one 64-partition copy + one min + one max. The level-B
    # mirror pairs (s0,s3),(s1,s2) = (Q0,Q3),(Q2,Q1) and stays as 32-part ops.
    def wide_stage(mirrored):
        # Co-located cross stage: stage the partner half into cols [L, 2L)
        # of cur on parts 0:64, run ONE flat-[2L] hijack op (halves pairing
        # (k, L+k): min -> col k, max -> col L+k), then copy the max half
        # back to parts 64:128 with the original orientation.
        nonlocal cur, nxt
        src = cur[64:128, L - 1 :: -1] if mirrored else cur[64:128, 0:L]
        nc.vector.tensor_copy(out=cur[0:64, L : 2 * L], in_=src)
        nc.vector.tensor_scalar_add(nxt[0:64, 0 : 2 * L], cur[0:64, 0 : 2 * L], 0.0)
        msrc = (
            nxt[0:64, 2 * L - 1 : L - 1 : -1] if mirrored else nxt[0:64, L : 2 * L]
        )
        nc.vector.tensor_copy(out=nxt[64:128, 0:L], in_=msrc)
        cur, nxt = nxt, cur

    def b_mirror_stage():
        # pairs: s0@[0:32] <-> s3@[96:128] rev;  s1@[64:96] <-> s2@[32:64] rev
        nonlocal cur, nxt
        nc.vector.tensor_copy(out=cur[0:32, L : 2 * L], in_=cur[96:128, L - 1 :: -1])
        nc.vector.tensor_copy(out=cur[64:96, L : 2 * L], in_=cur[32:64, L - 1 :: -1])
        # one hijack over parts 0:96: parts 32:64 compute garbage (their
        # staging cols hold wide_A leftovers) that the s2 copy-back replaces
        nc.vector.tensor_scalar_add(nxt[0:96, 0 : 2 * L], cur[0:96, 0 : 2 * L], 0.0)
        nc.vector.tensor_copy(
            out=nxt[96:128, 0:L], in_=nxt[0:32, 2 * L - 1 : L - 1 : -1]
        )
        nc.vector.tensor_copy(
            out=nxt[32:64, 0:L], in_=nxt[64:96, 2 * L - 1 : L - 1 : -1]
        )
        cur, nxt = nxt, cur

    # Level A: merge seg pairs (0,1) and (2,3) -> runs of 2L
    wide_stage(mirrored=True)
    uniform_stages(L // 2)

    # Level B: merge (seg0,seg1) with (seg2,seg3) -> full row sorted
    b_mirror_stage()
    wide_stage(mirrored=False)
    uniform_stages(L // 2)

    return cur


# ---------------------------------------------------------------------------
# per-core kernel
# ---------------------------------------------------------------------------
def emit_core_kernel(
    nc, tc, xt, ntot, out, dve_frac=1.0, cross_dve_frac=1.0, repeat=1, loop_n=1,
    fused=True, no_compute=False, no_dma=False,
):
    with ExitStack() as ctx:
        pool = ctx.enter_context(tc.tile_pool(name="main", bufs=1))
        X = pool.tile([128, 2 * L], FP32, tag="X")
        Y = pool.tile([128, 2 * L], FP32, tag="Y")
        XB = pool.tile([128, 2 * L], FP32, tag="XB", name="XB")
        YB = pool.tile([128, 2 * L], FP32, tag="YB", name="YB")
        bcolB = pool.tile([128, 1], FP32, tag="bcolB", name="bcolB")
        thr = pool.tile([128, 1], FP32, tag="thr")
        thr2 = pool.tile([128, 1], FP32, tag="thr2")
        nti = pool.tile([128, 1], mybir.dt.int32, tag="nti")
        offs = pool.tile([128, 1], FP32, tag="offs")
        bcol = pool.tile([128, 1], FP32, tag="bcol")

        # thr[p] = N_total[r] + 4 - 1024*s  (mask threshold vs column index);
        # thr2 = thr - 1023 for the segment-boundary column. Staged on Pool
        # (SWDGE for the tiny N_total loads) so the HWDGE queues are free
        # for the input pieces.
        for s in range(4):
            nc.gpsimd.dma_start(out=nti[QP[s] : QP[s] + 32, :], in_=ntot[:, :])
            nc.gpsimd.memset(offs[QP[s] : QP[s] + 32, :], float(4 - L * s))
        nc.gpsimd.tensor_copy(out=thr[:, :], in_=nti[:, :])
        nc.gpsimd.tensor_add(out=thr[:, :], in0=thr[:, :], in1=offs[:, :])
        nc.gpsimd.tensor_scalar_add(thr2[:, :], thr[:, :], float(-(L - 1)))

        def body(X, Y, bcol):
            if no_compute:
                # DMA in + out only
                nc.sync.dma_start(
                    out=X[:, 0:800],
                    in_=xt[0:4, :, :].rearrange("k r t -> (k r) t"),
                )
                for q in range(3):
                    nc.scalar.dma_start(
                        out=X[32 * q : 32 * q + 32, 800:1024],
                        in_=xt[4, :, 224 * q : 224 * (q + 1)],
                    )
                nc.scalar.dma_start(
                    out=X[96:128, 800:928], in_=xt[4, :, 672:800]
                )
                nc.sync.dma_start(out=out[:, :], in_=X[:, 0:L])
                return
            if no_dma:
                S = emit_sort(nc, X, Y, L)
                G = Y if S is X else X
                nc.vector._custom_dve(
                    MASKED_DIFF,
                    out=G[:, 0 : L - 1],
                    in0=S[:, 1:L],
                    in1=S[:, 0 : L - 1],
                    s0=thr[:, :],
                )
                return
            # The sort is input-order invariant, so the reference's concat
            # order is irrelevant — place band k at partitions [32k, 32k+32)
            # cols 0:800 (affine: ONE 128-partition DMA covering bands 0-3,
            # engaging all 16 SBUF DMA ports), and split band 4 across the
            # quadrants' cols 800:1024 (two more affine transfers).
            nc.scalar.dma_start(
                out=X[:, 0:800],
                in_=xt[0:4, :, :].rearrange("k r t -> (k r) t"),
            )
            for q in range(3):
                nc.scalar.dma_start(
                    out=X[32 * q : 32 * q + 32, 800:1024],
                    in_=xt[4, :, 224 * q : 224 * (q + 1)],
                )
            nc.scalar.dma_start(out=X[96:128, 800:928], in_=xt[4, :, 672:800])
            nc.gpsimd.memset(X[96:128, 928:1024], BIG)

            S = emit_sort(
                nc, X, Y, L, dve_frac=dve_frac, cross_dve_frac=cross_dve_frac,
                fused=fused,
            )
            G = Y if S is X else X

            # ---- fused masked diff: G[p,j] = (j < thr) ? S[j+1]-S[j] : 0 -
            # (measured per-instruction overhead ~0.5us outweighs any
            # split-and-overlap of the output DMA: keep ONE op + ONE DMA)
            nc.vector._custom_dve(
                MASKED_DIFF,
                out=G[:, 0 : L - 1],
                in0=S[:, 1:L],
                in1=S[:, 0 : L - 1],
                s0=thr[:, :],
            )
            # segment-boundary column: for s<3, G[QP[s]+r, 1023] =
            # masked(S[QP[s+1]+r, 0] - S[QP[s]+r, 1023]). With QP=[0,64,32,96]
            # the three next-seg staging copies merge into two:
            #   bcol[0:64]  <- S[64:128, 0]   (segs 0,2 partners)
            #   bcol[64:96] <- S[32:64, 0]    (seg 1 partner)
            nc.vector.tensor_copy(out=bcol[0:64, :], in_=S[64:128, 0:1])
            nc.vector.tensor_copy(out=bcol[64:96, :], in_=S[32:64, 0:1])
            nc.vector._custom_dve(
                MASKED_DIFF,
                out=G[0:96, L - 1 : L],
                in0=bcol[0:96, :],
                in1=S[0:96, L - 1 : L],
                s0=thr2[0:96, :],
            )
            # G[QP[3].., 1023] is never read by the host; zero it once so
            # the output buffer is deterministic.
            nc.gpsimd.memset(G[QP[3] : QP[3] + 32, L - 1 : L], 0.0)

            nc.sync.dma_start(out=out[:, :], in_=G[:, 0:L])

        if loop_n > 1:
            # double-buffered: two tile sets per For_i body so iteration
            # i+1's input DMA overlaps iteration i's compute
            assert loop_n % 16 == 0
            with tc.For_i(0, loop_n // 16, 1):
                for _ in range(8):
                    body(X, Y, bcol)
                    body(XB, YB, bcolB)
        else:
            for _ in range(repeat):
                body(X, Y, bcol)


def build_spmd_nc(dve_frac=1.0, cross_dve_frac=1.0, repeat=1, loop_n=1, fused=True,
                  no_compute=False, no_dma=False):
    nc = bacc.Bacc("TRN2", target_bir_lowering=False, debug=False)
    xt = nc.dram_tensor(
        "input_times", [NBANDS, ROWS, BANDLEN], FP32, kind="ExternalInput"
    )
    ntot = nc.dram_tensor("n_total", [ROWS, 1], mybir.dt.int32, kind="ExternalInput")
    out = nc.dram_tensor("out", [128, L], FP32, kind="ExternalOutput")
    with TileContext(nc) as tc:
        emit_core_kernel(
            nc,
            tc,
            xt[:, :, :],
            ntot[:, :],
            out[:, :],
            dve_frac=dve_frac,
            cross_dve_frac=cross_dve_frac,
            repeat=repeat,
            loop_n=loop_n,
            fused=fused,
            no_compute=no_compute,
            no_dma=no_dma,
        )
    # make table-gen include the hijacked TENSOR_SCALAR row in this NEFF
    nc.m.ant_custom_dve_ops = sorted({*nc.m.ant_custom_dve_ops, HIJACK_NAME})
    nc.compile()
    return nc


# ---------------------------------------------------------------------------
# host entry
# ---------------------------------------------------------------------------
_NC_CACHE = {}
_EXEC_CACHE = {}


def _get_exec(key, nc):
    """Build (once) a jitted SPMD executor for `nc` across 8 cores.

    Mirrors concourse.bass2jax.run_bass_via_pjrt's multi-core path, but
    caches the jax.jit wrapper so repeat invocations don't recompile."""
    if key in _EXEC_CACHE:
        return _EXEC_CACHE[key]

    import jax
    import concourse.mybir as _mybir
    from jax.sharding import Mesh, PartitionSpec
    from jax.experimental.shard_map import shard_map
    from concourse import bass2jax

    bass2jax.install_neuronx_cc_hook()

    in_names, out_names, out_avals, zero_outs = [], [], [], []
    partition_name = nc.partition_id_tensor.name if nc.partition_id_tensor else None
    for alloc in nc.m.functions[0].allocations:
        if not isinstance(alloc, _mybir.MemoryLocationSet):
            continue
        name = alloc.memorylocations[0].name
        if alloc.kind == "ExternalInput":
            if name != partition_name:
                in_names.append(name)
        elif alloc.kind == "ExternalOutput":
            shape = tuple(alloc.tensor_shape)
            dtype = _mybir.dt.np(alloc.dtype)
            out_names.append(name)
            out_avals.append(jax.core.ShapedArray(shape, dtype))
            zero_outs.append(np.zeros(shape, dtype))
    n_params = len(in_names)
    n_outs = len(out_avals)
    all_in_names = list(in_names) + list(out_names)
    if partition_name is not None:
        all_in_names.append(partition_name)
    donate = tuple(range(n_params, n_params + n_outs))

    def _body(*args):
        operands = list(args)
        if partition_name is not None:
            operands.append(bass2jax.partition_id_tensor())
        outs = bass2jax._bass_exec_p.bind(
            *operands,
            out_avals=tuple(out_avals),
            in_names=tuple(all_in_names),
            out_names=tuple(out_names),
            lowering_input_output_aliases=(),
            sim_require_finite=True,
            sim_require_nnan=True,
            nc=nc,
        )
        return tuple(outs)

    devices = jax.devices()[:N_CORES]
    mesh = Mesh(np.asarray(devices), ("core",))
    in_specs = (PartitionSpec("core"),) * (n_params + n_outs)
    out_specs = (PartitionSpec("core"),) * n_outs
    sharded = jax.jit(
        shard_map(
            _body, mesh=mesh, in_specs=in_specs, out_specs=out_specs, check_rep=False
        ),
        donate_argnums=donate,
        keep_unused=True,
    )

    def run(in_maps):
        concat_in = [
            np.concatenate([np.asarray(m[name]) for m in in_maps], axis=0)
            for name in in_names
        ]
        concat_zeros = [
            np.zeros((N_CORES * z.shape[0], *z.shape[1:]), z.dtype) for z in zero_outs
        ]
        out_arrs = sharded(*concat_in, *concat_zeros)
        return [
            {
                name: np.asarray(out_arrs[i]).reshape(N_CORES, *out_avals[i].shape)[c]
                for i, name in enumerate(out_names)
            }
            for c in range(N_CORES)
        ]

    _EXEC_CACHE[key] = run
    return run


def _get_nc(dve_frac, cross_dve_frac, repeat=1, loop_n=1, fused=True):
    key = (dve_frac, cross_dve_frac, repeat, loop_n)
    if key not in _NC_CACHE:
        _NC_CACHE[key] = build_spmd_nc(dve_frac, cross_dve_frac, repeat, loop_n)
    return _NC_CACHE[key]


def _run(input_times, N_total, dve_frac=1.0, cross_dve_frac=1.0, trace=False, repeat=1):
    input_times = np.ascontiguousarray(np.asarray(input_times, dtype=np.float32))
    N_total = np.asarray(N_total).astype(np.int32)
    assert input_times.shape == (NBANDS, 256, BANDLEN)
    assert N_total.shape == (256,)

    nc = _get_nc(dve_frac, cross_dve_frac, repeat)
    in_maps = []
    for c in range(N_CORES):
        rows = slice(c * ROWS, (c + 1) * ROWS)
        in_maps.append(
            {
                "input_times": np.ascontiguousarray(input_times[:, rows, :]),
                "n_total": np.ascontiguousarray(N_total[rows].reshape(ROWS, 1)),
            }
        )
    run = _get_exec((dve_frac, cross_dve_frac, repeat), nc)
    results = run(in_maps)
    outs = []
    for c in range(N_CORES):
        g = results[c]["out"]
        outs.append(
            np.concatenate(
                [
                    g[QP[0] : QP[0] + 32, 4:1024],
                    g[QP[1] : QP[1] + 32, :],
                    g[QP[2] : QP[2] + 32, :],
                    g[QP[3] : QP[3] + 32, 0:927],
                ],
                axis=1,
            )
        )
    full = np.concatenate(outs, axis=0).reshape(256, NOUT, 1).astype(np.float32)
    return full, None


def kernel(input_times, N_total):
    out, _ = _run(input_times, N_total)
    return out



# revision 27
# speedup vs baseline: 1.1055x; 1.1055x over previous
"""Trainium2 Bass kernel for nn_AllTimes (sort 4000 times/row -> adjacent
diffs -> mask by N_total).

Self-contained: kernel(**inputs) takes the FULL inputs
  input_times [5, 256, 800] f32, N_total [256] int (int32/int64)
and returns the FULL output [256, 3995, 1] f32.

Strategy: pure data parallel across 8 NeuronCores (32 batch rows each).
Per core, the 32 rows' 4000 values (padded to 4096 with BIG) are laid out
as [128 partitions, 1024] f32 (partition QP[s] + r holds row r's segment s)
and sorted with a bitonic merge network (78 half-stages) at TWO elements
per cycle per lane: the stock TENSOR_SCALAR_ARITH_OP (0x43) DVE table row
is overridden so its 2X_2PORT perf-mode slot holds a pairwise
compare-exchange uop program (port0 = first half of the AP stream, port1 =
second half; min -> WR0/port0 position, max -> WR1/port1 position). A
single-src fp32 SBUF->SBUF tensor_scalar auto-selects 2X_2PORT, so one
instruction with APs rearranged "p (b two j) -> p two b j" executes one
half-stage at stride j in ~(j_total/2)/0.96GHz. Mirror stages are
eliminated by a descending-odd-runs invariant: each level's closing j=1
stage writes odd runs descending (two half-width ops via the
flat-reversed view), making every level's first stage a straight riffle
on an asc-desc (bitonic) input. Cross-segment stages co-locate pair
members in cols [L, 2L) of the same partitions (copy + one flat-[2L]
hijack op + copy-back). Adjacent diffs and the N_total mask are fused
on-device (MASKED_DIFF custom op); the host only reshapes/concatenates
shards. The timing loop double-buffers two tile sets (16-body unroll)
with input DMAs on the ACT queue and output DMA on the SP queue so all
DMA hides behind DVE compute.
"""

import sys

sys.path.insert(0, "/opt/trn_rl_repo")

from contextlib import ExitStack

import numpy as np

import concourse.bass as bass
import concourse.bacc as bacc
import concourse.mybir as mybir
from concourse.tile import TileContext
from concourse import bass_utils

FP32 = mybir.dt.float32
AL = mybir.AluOpType
MIN = AL.min
MAX = AL.max


# ---------------------------------------------------------------------------
# fused compare-exchange custom DVE op:
#   out[k] = k < s0 ? min(Src0[k], Src1[k]) : max(Src0[k], Src1[k])
# One instruction computes both halves of a bitonic half-stage (for the
# stage shapes whose APs fit the custom-op 2-free-dim limit).
# ---------------------------------------------------------------------------
def _register_cmpxchg():
    import concourse.dve_ops as dve_ops
    from concourse.dve_spec import Spec, Src0, Src1, C0, Idx, minn, maxx, select, lower
    from concourse.dve_uop import DveOpSpec

    name = "CMPXCHG_HALVES_ANT"
    if name in dve_ops._SUB_OPCODE_FOR_NAME:
        return next(op for op in dve_ops.OPS if op.name == name)

    def _ref(in0, in1, s0, s1, imm2):
        x0 = np.asarray(in0, dtype=np.float32)
        x1 = np.asarray(in1, dtype=np.float32)
        P = x0.shape[0]
        f0 = x0.reshape(P, -1)
        f1 = x1.reshape(P, -1)
        idx = np.arange(f0.shape[1], dtype=np.float32)[None, :]
        thr = np.asarray(s0, dtype=np.float32).reshape(-1, 1)
        r = np.where(idx < thr, np.minimum(f0, f1), np.maximum(f0, f1))
        return r.reshape(x0.shape).astype(np.float32)

    spec = Spec(
        body=select(Idx < C0, minn(Src0, Src1), maxx(Src0, Src1)), reference=_ref
    )
    opcode = dve_ops._CUSTOM_DVE_ROW_BASE + len(dve_ops.OPS)
    shas = {}
    for ver in ("v3", "v4"):
        try:
            shas[ver] = DveOpSpec(
                name=name, opcode=opcode, uops=lower(spec, ver=ver), rd1_en=True
            ).sha(ver)
        except Exception:
            pass
    op = dve_ops.DveOp(name, spec, subdim=False, uops_sha=shas)
    dve_ops.OPS.append(op)
    dve_ops.CUSTOM_DVE_SPECS[name] = spec
    dve_ops._SUB_OPCODE_FOR_NAME[name] = opcode
    return op


CMPXCHG = _register_cmpxchg()


def _register_mirror_paged():
    """Paged mirror compare-exchange: for [P, nb, tm] streams,
    out[k] = (Idx < m + page*tm) ? min : max — i.e. within each tm-page,
    first half mins, second half maxes. s0 = m, s1 = tm."""
    import concourse.dve_ops as dve_ops
    from concourse.dve_spec import (
        Spec,
        Src0,
        Src1,
        C0,
        C1,
        Idx,
        PageIdx,
        minn,
        maxx,
        select,
        lower,
    )
    from concourse.dve_uop import DveOpSpec

    name = "CMPX_MIRROR_PAGED_ANT"
    if name in dve_ops._SUB_OPCODE_FOR_NAME:
        return next(op for op in dve_ops.OPS if op.name == name)

    def _ref(in0, in1, s0, s1, imm2):
        x0 = np.asarray(in0, dtype=np.float32)
        x1 = np.asarray(in1, dtype=np.float32)
        P = x0.shape[0]
        tm = int(x0.shape[-1])
        f0 = x0.reshape(P, -1, tm)
        f1 = x1.reshape(P, -1, tm)
        m = int(np.asarray(s0).flat[0])
        k = np.arange(tm)[None, None, :]
        r = np.where(k < m, np.minimum(f0, f1), np.maximum(f0, f1))
        return r.reshape(x0.shape).astype(np.float32)

    spec = Spec(
        body=select(
            Idx < PageIdx(C0, C1), minn(Src0, Src1), maxx(Src0, Src1)
        ),
        reference=_ref,
    )
    opcode = dve_ops._CUSTOM_DVE_ROW_BASE + len(dve_ops.OPS)
    shas = {}
    for ver in ("v3", "v4"):
        try:
            shas[ver] = DveOpSpec(
                name=name, opcode=opcode, uops=lower(spec, ver=ver), rd1_en=True
            ).sha(ver)
        except Exception:
            pass
    op = dve_ops.DveOp(name, spec, subdim=True, uops_sha=shas)
    dve_ops.OPS.append(op)
    dve_ops.CUSTOM_DVE_SPECS[name] = spec
    dve_ops._SUB_OPCODE_FOR_NAME[name] = opcode
    return op


MIRROR_PAGED = _register_mirror_paged()


def _register_masked_diff():
    """Fused tail op: out[k] = (Idx < s0[p]) ? (Src0[k] - Src1[k]) : 0."""
    import concourse.dve_ops as dve_ops
    from concourse.dve_spec import Spec, Src0, Src1, C0, Idx, Zero, select, lower
    from concourse.dve_uop import DveOpSpec

    name = "MASKED_DIFF_ANT"
    if name in dve_ops._SUB_OPCODE_FOR_NAME:
        return next(op for op in dve_ops.OPS if op.name == name)

    def _ref(in0, in1, s0, s1, imm2):
        x0 = np.asarray(in0, dtype=np.float32)
        x1 = np.asarray(in1, dtype=np.float32)
        P = x0.shape[0]
        f0 = x0.reshape(P, -1)
        f1 = x1.reshape(P, -1)
        idx = np.arange(f0.shape[1], dtype=np.float32)[None, :]
        thr = np.asarray(s0, dtype=np.float32).reshape(-1, 1)
        r = np.where(idx < thr, f0 - f1, np.float32(0.0))
        return r.reshape(x0.shape).astype(np.float32)

    spec = Spec(body=select(Idx < C0, Src0 - Src1, Zero), reference=_ref)
    opcode = dve_ops._CUSTOM_DVE_ROW_BASE + len(dve_ops.OPS)
    shas = {}
    for ver in ("v3", "v4"):
        try:
            shas[ver] = DveOpSpec(
                name=name, opcode=opcode, uops=lower(spec, ver=ver), rd1_en=True
            ).sha(ver)
        except Exception:
            pass
    op = dve_ops.DveOp(name, spec, subdim=False, uops_sha=shas)
    dve_ops.OPS.append(op)
    dve_ops.CUSTOM_DVE_SPECS[name] = spec
    dve_ops._SUB_OPCODE_FOR_NAME[name] = opcode
    return op


MASKED_DIFF = _register_masked_diff()


# ---------------------------------------------------------------------------
# Hijacked TENSOR_SCALAR_ARITH_OP (0x43) row: the 2X_2PORT perf-mode slot
# holds a pairwise compare-exchange program. A single-src fp32 SBUF->SBUF
# tensor_scalar auto-selects 2X_2PORT: port0 streams the first half of the
# AP stream, port1 the second half; per cycle the program computes
# min(a,b) -> WR0 (port0 position) and max(a,b) -> WR1 (port1 position).
# With APs rearranged "p (b two j) -> p two b j" this is one bitonic
# half-stage at 2 elements/cycle (~594ns for 1024 cols vs ~1224ns at 1x).
# The REGULAR slot is a plain copy so an (unexpected) fallback is caught by
# the correctness check rather than producing garbage.
# ---------------------------------------------------------------------------
HIJACK_NAME = "TS_CMPXCHG_HIJACK_ANT"
TS_OPCODE = 0x43  # TENSOR_SCALAR_ARITH_OP


def _register_ts_hijack():
    import concourse.dve_ops as dve_ops
    from concourse.dve_spec import Spec, Src0
    from concourse.dve_uop import (
        UopConfig,
        DveOpSpec,
        AluOp,
        AluInp,
        DelayInp,
        InpSel,
        OutSel,
        OutPath,
        Trigger,
        ENABLE,
    )

    if any(op.name == HIJACK_NAME for op in dve_ops.OPS):
        return

    def regular_copy():
        u = UopConfig()
        u.enable_input(InpSel.SRC_0, 0)
        u.require_inp0 = ENABLE
        u.trigger = (Trigger.SRC_TENSOR_DONE, Trigger.NONE, Trigger.NONE)
        u.enable_rev_ops = ENABLE
        for b in range(8):
            u.datapath_config[b].pass_through_alu()
        u.enable_output(OutSel.ALU_OUT, OutPath.WR0_LO)
        return u

    def pairwise():
        u = UopConfig()
        u.enable_input(InpSel.SRC_0, 0)
        u.enable_input(InpSel.SRC_1, 1)
        u.require_inp0 = ENABLE
        u.require_inp1 = ENABLE
        u.trigger = (Trigger.SRC_TENSOR_DONE, Trigger.NONE, Trigger.NONE)
        u.enable_rev_ops = ENABLE
        dp = u.datapath_config
        dp[0].enable_alu(AluOp.MIN, AluInp.PREV_ALU_OUT, AluInp.PREV_DELAY_0)
        dp[0].enable_delay_from_src(DelayInp.PREV_DELAY, 0)  # chain0 <- src1
        dp[0].enable_delay_from_src(DelayInp.PREV_ALU_OUT, 1)  # chain1 <- src0
        dp[1].enable_alu(AluOp.MAX, AluInp.PREV_DELAY_1, AluInp.PREV_DELAY_0)
        dp[1].enable_delay_from_src(DelayInp.PREV_ALU_OUT, 2)  # chain2 <- min
        dp[2].pass_through_delay(2)
        dp[2].enable_delay_from_src(DelayInp.PREV_ALU_OUT, 3)  # chain3 <- max
        for b in range(3, 8):
            dp[b].pass_through_delay(2, 3)
        u.enable_output(OutSel.DELAY_2, OutPath.WR0_LO)  # min
        u.enable_output(OutSel.DELAY_3, OutPath.WR1_LO)  # max
        return u

    spec = DveOpSpec(
        name=HIJACK_NAME,
        opcode=TS_OPCODE,
        uops=[regular_copy()],
        uops_2x=[regular_copy()],
        uops_2x_2p=[pairwise()],
        uops_4x=None,
    )

    def _ref(in0, in1, s0, s1, imm2):
        return np.asarray(in0, dtype=np.float32)

    entry = dve_ops.DveOp(
        HIJACK_NAME, Spec(body=Src0, reference=_ref), subdim=False, uops_sha={}
    )
    dve_ops.OPS.append(entry)
    dve_ops.CUSTOM_DVE_SPECS[HIJACK_NAME] = entry.spec
    for ver in ("v3", "v4"):
        dve_ops._COMPILE_CACHE[(HIJACK_NAME, ver)] = spec


_register_ts_hijack()

N_CORES = 8
NBANDS = 5
BANDLEN = 800
ROWS = 32  # batch rows per core
L = 1024  # elements per partition (segment length); 4 segs per row
NOUT = 3995
BIG = 3.0e38

# Logical segment s lives at partition offset QP[s] (quadrant placement
# [s0, s2, s1, s3]) so that the level-A and level-B-stride cross-segment
# half-stages become single 64-partition ops.
QP = [0, 64, 32, 96]

# (band k, t0, t1, seg s, j0): input_times[k, :, t0:t1] -> X[QP[s]:+32, j0:...]
PIECES = [
    (0, 0, 800, 0, 0),
    (1, 0, 224, 0, 800),
    (1, 224, 800, 1, 0),
    (2, 0, 448, 1, 576),
    (2, 448, 800, 2, 0),
    (3, 0, 672, 2, 352),
    (3, 672, 800, 3, 0),
    (4, 0, 800, 3, 128),
]


def _emit_split(nc, dve_frac, op, make_aps, width):
    """One logical compare op, split by columns across DVE and Pool."""
    c = int(round(width * dve_frac))
    c = max(0, min(width, c))
    if c > 0:
        out, a, b = make_aps(0, c)
        nc.vector.tensor_tensor(out=out, in0=a, in1=b, op=op)
    if c < width:
        out, a, b = make_aps(c, width)
        nc.gpsimd.tensor_tensor(out=out, in0=a, in1=b, op=op)


def _rev(base, lo, hi):
    stop = base - hi
    return slice(base - lo, (stop if stop >= 0 else None), -1)


def _down_powers(start, stop):
    j = start
    while j >= stop:
        yield j
        j //= 2


def emit_sort(nc, cur, nxt, seglen, dve_frac=1.0, cross_dve_frac=1.0, fused=True):
    """Sort network over [128, seglen] ping-pong tiles cur/nxt; T is a
    [128, seglen] scratch tile for cross-segment operand alignment.
    Returns the tile holding the sorted result."""
    L = seglen
    nlev = L.bit_length() - 1
    H = L // 2

    def halfstage(ops):
        nonlocal cur, nxt
        for op, make, width in ops:
            _emit_split(nc, dve_frac, op, make, width)
        cur, nxt = nxt, cur

    def fused_j1():
        # one instruction: mins to even cols, maxes to odd cols
        nonlocal cur, nxt
        lo = cur[:, 0::2].unsqueeze(1).to_broadcast((128, 2, H))
        hi = cur[:, 1::2].unsqueeze(1).to_broadcast((128, 2, H))
        o = nxt[:, :].rearrange("p (b two) -> p two b", two=2)
        nc.vector._custom_dve(CMPXCHG, out=o, in0=lo, in1=hi, s0=float(H))
        cur, nxt = nxt, cur

    def fused_j_half():
        # stride L/2 (single block): mins to cols [0,H), maxes to [H,L)
        nonlocal cur, nxt
        lo = cur[:, 0:H].unsqueeze(1).to_broadcast((128, 2, H))
        hi = cur[:, H:L].unsqueeze(1).to_broadcast((128, 2, H))
        nc.vector._custom_dve(CMPXCHG, out=nxt[:, :], in0=lo, in1=hi, s0=float(H))
        cur, nxt = nxt, cur

    def riffle_stage(jj, parts=slice(0, 128)):
        # One compare-exchange half-stage at stride jj as a single hijacked
        # tensor_scalar: halves-major AP [p, two, b, j]; 2X_2PORT computes
        # min -> lo position, max -> hi position at 2 elems/cycle.
        nonlocal cur, nxt
        if jj == 1:
            i_ap = cur[parts, 0:L].rearrange("p (b two) -> p two b", two=2)
            o_ap = nxt[parts, 0:L].rearrange("p (b two) -> p two b", two=2)
        else:
            i_ap = cur[parts, 0:L].rearrange(
                "p (b two j) -> p two b j", two=2, j=jj
            )
            o_ap = nxt[parts, 0:L].rearrange(
                "p (b two j) -> p two b j", two=2, j=jj
            )
        nc.vector.tensor_scalar_add(o_ap, i_ap, 0.0)
        cur, nxt = nxt, cur

    def split_j1_stage(m2):
        # Last (j=1) stage of the level finalizing m2-runs: write ODD m2-runs
        # descending so the next level's first stage is a straight riffle
        # (asc‖desc is bitonic). Two half-width ops: even runs via the normal
        # view, odd runs via the flat-reversed view (odd-run data appears at
        # the reversed view's r2=0 slot, and min->higher-address = descending
        # storage falls out of the same construction).
        nonlocal cur, nxt
        q = max(1, m2 // 2)
        ci = cur[:, 0:L].rearrange(
            "p (rh r2 q two) -> p r2 two rh q", r2=2, two=2, q=q
        )
        no = nxt[:, 0:L].rearrange(
            "p (rh r2 q two) -> p r2 two rh q", r2=2, two=2, q=q
        )
        rno = nxt[:, L - 1 :: -1].rearrange(
            "p (rh r2 q two) -> p r2 two rh q", r2=2, two=2, q=q
        )
        # even runs: normal pairs, ascending write
        nc.vector.tensor_scalar_add(no[:, 0], ci[:, 0], 0.0)
        # odd runs: input pairs with rh reversed (to match the reversed-view
        # output enumeration); min lands at local m2-1-2k => descending run
        nc.vector.tensor_scalar_add(rno[:, 0], ci[:, 1, :, ::-1, :], 0.0)
        cur, nxt = nxt, cur

    def uniform_stages(first_j):
        j = first_j
        while j >= 1:
            riffle_stage(j)
            j //= 2

    # ---- Phase 1: in-partition sort to runs of L -------------------------
    # Descending-odd-runs invariant: the j=1 stage closing each level writes
    # odd runs descending (split_j1_stage), so every level's first stage is a
    # straight riffle at j=m (asc‖desc input is bitonic) — no mirror ops.
    # Level 10 closes with a normal j=1 stage; level A's copy handles the
    # cross-segment reversal as before.
    for lev in range(1, nlev + 1):
        m = 1 << (lev - 1)
        if lev > 1:
            riffle_stage(m)  # straight first stage on asc‖desc runs
        for j in _down_powers(m // 2, 2):
            riffle_stage(j)
        if lev < nlev:
            split_j1_stage(2 * m)
        else:
            riffle_stage(1)

    # ---- cross-segment half-stages ---------------------------------------
    # With the QP placement ([s0@Q0, s2@Q1, s1@Q2, s3@Q3]), the level-A
    # mirror and level-B straight stages pair partitions 0:64 with 64:128,
    # so each is one 64-partition copy + one min + one max. The level-B
    # mirror pairs (s0,s3),(s1,s2) = (Q0,Q3),(Q2,Q1) and stays as 32-part ops.
    def wide_stage(mirrored):
        # Co-located cross stage: stage the partner half into cols [L, 2L)
        # of cur on parts 0:64, run ONE flat-[2L] hijack op (halves pairing
        # (k, L+k): min -> col k, max -> col L+k), then copy the max half
        # back to parts 64:128 with the original orientation.
        nonlocal cur, nxt
        src = cur[64:128, L - 1 :: -1] if mirrored else cur[64:128, 0:L]
        nc.vector.tensor_copy(out=cur[0:64, L : 2 * L], in_=src)
        nc.vector.tensor_scalar_add(nxt[0:64, 0 : 2 * L], cur[0:64, 0 : 2 * L], 0.0)
        msrc = (
            nxt[0:64, 2 * L - 1 : L - 1 : -1] if mirrored else nxt[0:64, L : 2 * L]
        )
        nc.vector.tensor_copy(out=nxt[64:128, 0:L], in_=msrc)
        cur, nxt = nxt, cur

    def b_mirror_stage():
        # pairs: s0@[0:32] <-> s3@[96:128] rev;  s1@[64:96] <-> s2@[32:64] rev
        nonlocal cur, nxt
        nc.vector.tensor_copy(out=cur[0:32, L : 2 * L], in_=cur[96:128, L - 1 :: -1])
        nc.vector.tensor_copy(out=cur[64:96, L : 2 * L], in_=cur[32:64, L - 1 :: -1])
        # one hijack over parts 0:96: parts 32:64 compute garbage (their
        # staging cols hold wide_A leftovers) that the s2 copy-back replaces
        nc.vector.tensor_scalar_add(nxt[0:96, 0 : 2 * L], cur[0:96, 0 : 2 * L], 0.0)
        nc.vector.tensor_copy(
            out=nxt[96:128, 0:L], in_=nxt[0:32, 2 * L - 1 : L - 1 : -1]
        )
        nc.vector.tensor_copy(
            out=nxt[32:64, 0:L], in_=nxt[64:96, 2 * L - 1 : L - 1 : -1]
        )
        cur, nxt = nxt, cur

    # Level A: merge seg pairs (0,1) and (2,3) -> runs of 2L
    wide_stage(mirrored=True)
    uniform_stages(L // 2)

    # Level B: merge (seg0,seg1) with (seg2,seg3) -> full row sorted
    b_mirror_stage()
    wide_stage(mirrored=False)
    uniform_stages(L // 2)

    return cur


# ---------------------------------------------------------------------------
# per-core kernel
# ---------------------------------------------------------------------------
def emit_core_kernel(
    nc, tc, xt, ntot, out, dve_frac=1.0, cross_dve_frac=1.0, repeat=1, loop_n=1,
    fused=True, no_compute=False, no_dma=False,
):
    with ExitStack() as ctx:
        pool = ctx.enter_context(tc.tile_pool(name="main", bufs=1))
        X = pool.tile([128, 2 * L], FP32, tag="X")
        Y = pool.tile([128, 2 * L], FP32, tag="Y")
        XB = pool.tile([128, 2 * L], FP32, tag="XB", name="XB")
        YB = pool.tile([128, 2 * L], FP32, tag="YB", name="YB")
        bcolB = pool.tile([128, 1], FP32, tag="bcolB", name="bcolB")
        thr = pool.tile([128, 1], FP32, tag="thr")
        thr2 = pool.tile([128, 1], FP32, tag="thr2")
        nti = pool.tile([128, 1], mybir.dt.int32, tag="nti")
        offs = pool.tile([128, 1], FP32, tag="offs")
        bcol = pool.tile([128, 1], FP32, tag="bcol")

        # thr[p] = N_total[r] + 4 - 1024*s  (mask threshold vs column index);
        # thr2 = thr - 1023 for the segment-boundary column. Staged on Pool
        # (SWDGE for the tiny N_total loads) so the HWDGE queues are free
        # for the input pieces.
        for s in range(4):
            nc.gpsimd.dma_start(out=nti[QP[s] : QP[s] + 32, :], in_=ntot[:, :])
            nc.gpsimd.memset(offs[QP[s] : QP[s] + 32, :], float(4 - L * s))
        nc.gpsimd.tensor_copy(out=thr[:, :], in_=nti[:, :])
        nc.gpsimd.tensor_add(out=thr[:, :], in0=thr[:, :], in1=offs[:, :])
        nc.gpsimd.tensor_scalar_add(thr2[:, :], thr[:, :], float(-(L - 1)))

        def body(X, Y, bcol):
            if no_compute:
                # DMA in + out only
                nc.sync.dma_start(
                    out=X[:, 0:800],
                    in_=xt[0:4, :, :].rearrange("k r t -> (k r) t"),
                )
                for q in range(3):
                    nc.scalar.dma_start(
                        out=X[32 * q : 32 * q + 32, 800:1024],
                        in_=xt[4, :, 224 * q : 224 * (q + 1)],
                    )
                nc.scalar.dma_start(
                    out=X[96:128, 800:928], in_=xt[4, :, 672:800]
                )
                nc.sync.dma_start(out=out[:, :], in_=X[:, 0:L])
                return
            if no_dma:
                S = emit_sort(nc, X, Y, L)
                G = Y if S is X else X
                nc.vector._custom_dve(
                    MASKED_DIFF,
                    out=G[:, 0 : L - 1],
                    in0=S[:, 1:L],
                    in1=S[:, 0 : L - 1],
                    s0=thr[:, :],
                )
                return
            # The sort is input-order invariant, so the reference's concat
            # order is irrelevant — place band k at partitions [32k, 32k+32)
            # cols 0:800 (affine: ONE 128-partition DMA covering bands 0-3,
            # engaging all 16 SBUF DMA ports), and split band 4 across the
            # quadrants' cols 800:1024 (two more affine transfers).
            nc.scalar.dma_start(
                out=X[:, 0:800],
                in_=xt[0:4, :, :].rearrange("k r t -> (k r) t"),
            )
            for q in range(3):
                nc.scalar.dma_start(
                    out=X[32 * q : 32 * q + 32, 800:1024],
                    in_=xt[4, :, 224 * q : 224 * (q + 1)],
                )
            nc.scalar.dma_start(out=X[96:128, 800:928], in_=xt[4, :, 672:800])
            nc.gpsimd.memset(X[96:128, 928:1024], BIG)

            S = emit_sort(
                nc, X, Y, L, dve_frac=dve_frac, cross_dve_frac=cross_dve_frac,
                fused=fused,
            )
            G = Y if S is X else X

            # ---- fused masked diff: G[p,j] = (j < thr) ? S[j+1]-S[j] : 0 -
            # (measured per-instruction overhead ~0.5us outweighs any
            # split-and-overlap of the output DMA: keep ONE op + ONE DMA)
            nc.vector._custom_dve(
                MASKED_DIFF,
                out=G[:, 0 : L - 1],
                in0=S[:, 1:L],
                in1=S[:, 0 : L - 1],
                s0=thr[:, :],
            )
            # segment-boundary column: for s<3, G[QP[s]+r, 1023] =
            # masked(S[QP[s+1]+r, 0] - S[QP[s]+r, 1023]). With QP=[0,64,32,96]
            # the three next-seg staging copies merge into two:
            #   bcol[0:64]  <- S[64:128, 0]   (segs 0,2 partners)
            #   bcol[64:96] <- S[32:64, 0]    (seg 1 partner)
            nc.vector.tensor_copy(out=bcol[0:64, :], in_=S[64:128, 0:1])
            nc.vector.tensor_copy(out=bcol[64:96, :], in_=S[32:64, 0:1])
            nc.vector._custom_dve(
                MASKED_DIFF,
                out=G[0:96, L - 1 : L],
                in0=bcol[0:96, :],
                in1=S[0:96, L - 1 : L],
                s0=thr2[0:96, :],
            )
            # G[QP[3].., 1023] is never read by the host; zero it once so
            # the output buffer is deterministic.
            nc.gpsimd.memset(G[QP[3] : QP[3] + 32, L - 1 : L], 0.0)

            nc.sync.dma_start(out=out[:, :], in_=G[:, 0:L])

        if loop_n > 1:
            # double-buffered: two tile sets per For_i body so iteration
            # i+1's input DMA overlaps iteration i's compute
            assert loop_n % 16 == 0
            with tc.For_i(0, loop_n // 16, 1):
                for _ in range(8):
                    body(X, Y, bcol)
                    body(XB, YB, bcolB)
        else:
            for _ in range(repeat):
                body(X, Y, bcol)


def build_spmd_nc(dve_frac=1.0, cross_dve_frac=1.0, repeat=1, loop_n=1, fused=True,
                  no_compute=False, no_dma=False):
    nc = bacc.Bacc("TRN2", target_bir_lowering=False, debug=False)
    xt = nc.dram_tensor(
        "input_times", [NBANDS, ROWS, BANDLEN], FP32, kind="ExternalInput"
    )
    ntot = nc.dram_tensor("n_total", [ROWS, 1], mybir.dt.int32, kind="ExternalInput")
    out = nc.dram_tensor("out", [128, L], FP32, kind="ExternalOutput")
    with TileContext(nc) as tc:
        emit_core_kernel(
            nc,
            tc,
            xt[:, :, :],
            ntot[:, :],
            out[:, :],
            dve_frac=dve_frac,
            cross_dve_frac=cross_dve_frac,
            repeat=repeat,
            loop_n=loop_n,
            fused=fused,
            no_compute=no_compute,
            no_dma=no_dma,
        )
    # make table-gen include the hijacked TENSOR_SCALAR row in this NEFF
    nc.m.ant_custom_dve_ops = sorted({*nc.m.ant_custom_dve_ops, HIJACK_NAME})
    nc.compile()
    return nc


# ---------------------------------------------------------------------------
# host entry
# ---------------------------------------------------------------------------
_NC_CACHE = {}
_EXEC_CACHE = {}


def _get_exec(key, nc):
    """Build (once) a jitted SPMD executor for `nc` across 8 cores.

    Mirrors concourse.bass2jax.run_bass_via_pjrt's multi-core path, but
    caches the jax.jit wrapper so repeat invocations don't recompile."""
    if key in _EXEC_CACHE:
        return _EXEC_CACHE[key]

    import jax
    import concourse.mybir as _mybir
    from jax.sharding import Mesh, PartitionSpec
    from jax.experimental.shard_map import shard_map
    from concourse import bass2jax

    bass2jax.install_neuronx_cc_hook()

    in_names, out_names, out_avals, zero_outs = [], [], [], []
    partition_name = nc.partition_id_tensor.name if nc.partition_id_tensor else None
    for alloc in nc.m.functions[0].allocations:
        if not isinstance(alloc, _mybir.MemoryLocationSet):
            continue
        name = alloc.memorylocations[0].name
        if alloc.kind == "ExternalInput":
            if name != partition_name:
                in_names.append(name)
        elif alloc.kind == "ExternalOutput":
            shape = tuple(alloc.tensor_shape)
            dtype = _mybir.dt.np(alloc.dtype)
            out_names.append(name)
            out_avals.append(jax.core.ShapedArray(shape, dtype))
            zero_outs.append(np.zeros(shape, dtype))
    n_params = len(in_names)
    n_outs = len(out_avals)
    all_in_names = list(in_names) + list(out_names)
    if partition_name is not None:
        all_in_names.append(partition_name)
    donate = tuple(range(n_params, n_params + n_outs))

    def _body(*args):
        operands = list(args)
        if partition_name is not None:
            operands.append(bass2jax.partition_id_tensor())
        outs = bass2jax._bass_exec_p.bind(
            *operands,
            out_avals=tuple(out_avals),
            in_names=tuple(all_in_names),
            out_names=tuple(out_names),
            lowering_input_output_aliases=(),
            sim_require_finite=True,
            sim_require_nnan=True,
            nc=nc,
        )
        return tuple(outs)

    devices = jax.devices()[:N_CORES]
    mesh = Mesh(np.asarray(devices), ("core",))
    in_specs = (PartitionSpec("core"),) * (n_params + n_outs)
    out_specs = (PartitionSpec("core"),) * n_outs
    sharded = jax.jit(
        shard_map(
            _body, mesh=mesh, in_specs=in_specs, out_specs=out_specs, check_rep=False
        ),
        donate_argnums=donate,
        keep_unused=True,
    )

    def run(in_maps):
        concat_in = [
            np.concatenate([np.asarray(m[name]) for m in in_maps], axis=0)
            for name in in_names
        ]
        concat_zeros = [
            np.zeros((N_CORES * z.shape[0], *z.shape[1:]), z.dtype) for z in zero_outs
        ]
        out_arrs = sharded(*concat_in, *concat_zeros)
        return [
            {
                name: np.asarray(out_arrs[i]).reshape(N_CORES, *out_avals[i].shape)[c]
                for i, name in enumerate(out_names)
            }
            for c in range(N_CORES)
        ]

    _EXEC_CACHE[key] = run
    return run


def _get_nc(dve_frac, cross_dve_frac, repeat=1, loop_n=1, fused=True):
    key = (dve_frac, cross_dve_frac, repeat, loop_n)
    if key not in _NC_CACHE:
        _NC_CACHE[key] = build_spmd_nc(dve_frac, cross_dve_frac, repeat, loop_n)
    return _NC_CACHE[key]


def _run(input_times, N_total, dve_frac=1.0, cross_dve_frac=1.0, trace=False, repeat=1):
    input_times = np.ascontiguousarray(np.asarray(input_times, dtype=np.float32))
    N_total = np.asarray(N_total).astype(np.int32)
    assert input_times.shape == (NBANDS, 256, BANDLEN)
    assert N_total.shape == (256,)

    nc = _get_nc(dve_frac, cross_dve_frac, repeat)
    in_maps = []
    for c in range(N_CORES):
        rows = slice(c * ROWS, (c + 1) * ROWS)
        in_maps.append(
            {
                "input_times": np.ascontiguousarray(input_times[:, rows, :]),
                "n_total": np.ascontiguousarray(N_total[rows].reshape(ROWS, 1)),
            }
        )
    run = _get_exec((dve_frac, cross_dve_frac, repeat), nc)
    results = run(in_maps)
    outs = []
    for c in range(N_CORES):
        g = results[c]["out"]
        outs.append(
            np.concatenate(
                [
                    g[QP[0] : QP[0] + 32, 4:1024],
                    g[QP[1] : QP[1] + 32, :],
                    g[QP[2] : QP[2] + 32, :],
                    g[QP[3] : QP[3] + 32, 0:927],
                ],
                axis=1,
            )
        )
    full = np.concatenate(outs, axis=0).reshape(256, NOUT, 1).astype(np.float32)
    return full, None


def kernel(input_times, N_total):
    out, _ = _run(input_times, N_total)
    return out

